# revision 3
# baseline (speedup 1.0000x reference)
"""Multi-head QKV block attention for Trainium2, SPMD over 8 NeuronCores.

Problem: X[4,2048,1024], residual[4,2048,1024], wq/wk/wv[1024,1024],
H=16 heads, D=64, softmax scale sqrt(S/H)=sqrt(128).
out = softmax((X wq)(X wk)^T / sqrt(128)) (X wv) + residual, returned twice.

Sharding: core c handles batch b=c//2 and head group g=c%2 (8 heads = 512
feature columns). Fully data/tensor-parallel -- no collectives; host
assembles the output.

Per-core kernel (Tile framework):
  B) X s-tiles are PE-transposed to X^T (f32r); Q^T,K^T projected with
     weights as stationary operands (stored bf16), V projected in natural
     layout (f32r) with a ones-column appended per head.
  C) per head pair: transposed logits K_h^T.T @ Q_h^T as two K=64 matmuls
     in disjoint PE row groups; exp on ScalarE (scale folded, no max
     subtraction -- logits are O(10) so fp32 exp is safe); effect^T
     accumulated as [v|1].T @ expT (ones row gives the softmax denominator);
     PE-transpose back, normalize, add residual, DMA out.
"""

import math
import sys

for _p in ("/opt/trn_rl_repo", "/root/.axon_site/_ro/trn_rl_repo"):
    if _p not in sys.path:
        sys.path.append(_p)

import numpy as np

B, S, F = 4, 2048, 1024
H = 16
D = 64
G = 512            # feature columns per core (8 heads)
NH = 8             # heads per core
KC = 8             # contraction chunks of 128 over F
ST = 4             # s tiles of 512
TC = 16            # t chunks of 128
SCALE = 1.0 / math.sqrt(S / H)

_cached = None


def _build():
    import concourse.bacc as bacc
    import concourse.tile as tile
    from concourse import mybir
    from concourse.masks import make_identity

    dt = mybir.dt
    AF = mybir.ActivationFunctionType

    nc = bacc.Bacc("TRN2", target_bir_lowering=False, debug=False, num_devices=8)

    x_d = nc.dram_tensor("x", [S, F], dt.float32, kind="ExternalInput").ap()
    wq_d = nc.dram_tensor("wq", [F, G], dt.float32, kind="ExternalInput").ap()
    wk_d = nc.dram_tensor("wk", [F, G], dt.float32, kind="ExternalInput").ap()
    wv_d = nc.dram_tensor("wv", [F, G], dt.float32, kind="ExternalInput").ap()
    res_d = nc.dram_tensor("res", [S, G], dt.float32, kind="ExternalInput").ap()
    out_d = nc.dram_tensor("out", [S, G], dt.float32, kind="ExternalOutput").ap()

    with tile.TileContext(nc) as tc:
        with tc.tile_pool(name="persist", bufs=1) as persist:
            ident = persist.tile([128, 128], dt.float32)
            make_identity(nc, ident[:])
            ones = persist.tile([128, NH], dt.float32)
            nc.vector.memset(ones[:], 1.0)
            qT = [persist.tile([128, S], dt.bfloat16, name=f"qT{m}") for m in range(4)]
            kT = [persist.tile([128, S], dt.bfloat16, name=f"kT{m}") for m in range(4)]
            vS = [persist.tile([128, NH, D + 1], dt.float32r, name=f"vS{t}")
                  for t in range(TC)]

            # ---- Phase B: transpose X, project Q^T, K^T, V ----
            with tc.tile_pool(name="wp", bufs=1) as wp, \
                 tc.tile_pool(name="xsp", bufs=6) as xsp, \
                 tc.tile_pool(name="xtp", bufs=9) as xtp, \
                 tc.tile_pool(name="psB", bufs=2, space="PSUM") as psB, \
                 tc.tile_pool(name="psP", bufs=2, space="PSUM") as psP:
                w_sb = {}
                for nm, wd in (("q", wq_d), ("k", wk_d), ("v", wv_d)):
                    for k in range(KC):
                        t = wp.tile([128, G], dt.float32r, name=f"w{nm}{k}")
                        nc.gpsimd.dma_start(t[:], wd[k * 128:(k + 1) * 128, :])
                        w_sb[nm, k] = t

                for st in range(ST):
                    s0 = st * 512
                    xs = []
                    for j in range(4):
                        t = xsp.tile([128, F], dt.float32, tag="xs", name="xs")
                        nc.sync.dma_start(t[:], x_d[s0 + j * 128:s0 + (j + 1) * 128, :])
                        xs.append(t)
                    xTs = []
                    for k in range(KC):
                        pt = psB.tile([128, 512], dt.float32, tag="xtp", name="pt")
                        for j in range(4):
                            nc.tensor.transpose(
                                pt[:, j * 128:(j + 1) * 128],
                                xs[j][:, k * 128:(k + 1) * 128], ident[:])
                        xt = xtp.tile([128, 512], dt.float32r, tag="xT", name="xt")
                        nc.vector.tensor_copy(xt[:], pt[:])
                        xTs.append(xt)
                    for m in range(4):
                        pq = psP.tile([128, 512], dt.float32, tag="proj", name="pq")
                        for k in range(KC):
                            nc.tensor.matmul(
                                pq[:], w_sb["q", k][:, m * 128:(m + 1) * 128],
                                xTs[k][:], start=(k == 0), stop=(k == KC - 1))
                        nc.vector.tensor_copy(qT[m][:, s0:s0 + 512], pq[:])
                    for m in range(4):
                        pk = psP.tile([128, 512], dt.float32, tag="proj", name="pk")
                        for k in range(KC):
                            nc.tensor.matmul(
                                pk[:], w_sb["k", k][:, m * 128:(m + 1) * 128],
                                xTs[k][:], start=(k == 0), stop=(k == KC - 1))
                        nc.vector.tensor_copy(kT[m][:, s0:s0 + 512], pk[:])
                    for j in range(4):
                        pv = psP.tile([128, 512], dt.float32, tag="proj", name="pv")
                        for k in range(KC):
                            nc.tensor.matmul(
                                pv[:], xTs[k][:, j * 128:(j + 1) * 128],
                                w_sb["v", k][:], start=(k == 0), stop=(k == KC - 1))
                        tci = st * 4 + j
                        nc.vector.tensor_copy(
                            vS[tci][:, :, D:D + 1],
                            ones[:].rearrange("p (h o) -> p h o", o=1))
                        nc.vector.tensor_copy(
                            vS[tci][:, :, 0:D],
                            pv[:].rearrange("p (h d) -> p h d", h=NH))

            # ---- Phase C: attention ----
            with tc.tile_pool(name="lpp", bufs=4, space="PSUM") as lpp, \
                 tc.tile_pool(name="epp", bufs=2, space="PSUM") as epp, \
                 tc.tile_pool(name="tpp", bufs=2, space="PSUM") as tpp, \
                 tc.tile_pool(name="expp", bufs=4) as expp, \
                 tc.tile_pool(name="esp", bufs=2) as esp, \
                 tc.tile_pool(name="stp", bufs=8) as stp, \
                 tc.tile_pool(name="rsp", bufs=3) as rsp, \
                 tc.tile_pool(name="rcp", bufs=4) as rcp:
                for st in range(ST):
                    s0 = st * 512
                    stage = [stp.tile([128, G], dt.float32, tag="stage", name="stage")
                             for _ in range(4)]
                    for hp in range(4):
                        eps = [epp.tile([D + 1, 512], dt.float32, tag="ep", name="ep")
                               for _ in range(2)]
                        for t in range(TC):
                            for half in range(2):
                                r0 = half * 64
                                lp = lpp.tile([128, 512], dt.float32, tag="lp", name="lp")
                                nc.tensor.matmul(
                                    lp[:],
                                    kT[hp][r0:r0 + 64, t * 128:(t + 1) * 128],
                                    qT[hp][r0:r0 + 64, s0:s0 + 512],
                                    start=True, stop=True)
                                ex = expp.tile([128, 512], dt.float32r, tag="exp", name="ex")
                                nc.scalar.activation(ex[:], lp[:], AF.Exp, scale=SCALE)
                                nc.tensor.matmul(
                                    eps[half][:], vS[t][:, 2 * hp + half, :], ex[:],
                                    start=(t == 0), stop=(t == TC - 1))
                        for half in range(2):
                            h = 2 * hp + half
                            es = esp.tile([D + 1, 512], dt.float32, tag="es", name="es")
                            nc.vector.tensor_copy(es[:], eps[half][:])
                            for j in range(4):
                                tp_t = tpp.tile([128, D + 1], dt.float32, tag="tp", name="tp")
                                nc.tensor.transpose(
                                    tp_t[:], es[:, j * 128:(j + 1) * 128],
                                    ident[0:D + 1, 0:D + 1])
                                rec = rcp.tile([128, 1], dt.float32, tag="rec", name="rec")
                                nc.vector.reciprocal(rec[:], tp_t[:, D:D + 1])
                                nc.vector.tensor_scalar_mul(
                                    stage[j][:, h * 64:(h + 1) * 64],
                                    tp_t[:, 0:D], rec[:])
                    for j in range(4):
                        rt = rsp.tile([128, G], dt.float32, tag="res", name="rt")
                        nc.sync.dma_start(
                            rt[:], res_d[s0 + j * 128:s0 + (j + 1) * 128, :])
                        nc.vector.tensor_add(stage[j][:], stage[j][:], rt[:])
                        nc.sync.dma_start(
                            out_d[s0 + j * 128:s0 + (j + 1) * 128, :], stage[j][:])

    nc.compile()
    return nc


def _get_nc():
    global _cached
    if _cached is None:
        _cached = _build()
    return _cached


def _make_in_maps(X, residual_score, wq, wk, wv):
    X = np.ascontiguousarray(np.asarray(X, dtype=np.float32))
    residual_score = np.ascontiguousarray(np.asarray(residual_score, dtype=np.float32))
    wq = np.ascontiguousarray(np.asarray(wq, dtype=np.float32))
    wk = np.ascontiguousarray(np.asarray(wk, dtype=np.float32))
    wv = np.ascontiguousarray(np.asarray(wv, dtype=np.float32))
    in_maps = []
    for c in range(8):
        b, g = c // 2, c % 2
        cols = slice(g * G, (g + 1) * G)
        in_maps.append({
            "x": X[b],
            "wq": np.ascontiguousarray(wq[:, cols]),
            "wk": np.ascontiguousarray(wk[:, cols]),
            "wv": np.ascontiguousarray(wv[:, cols]),
            "res": np.ascontiguousarray(residual_score[b, :, cols]),
        })
    return in_maps


def _assemble(results):
    out = np.empty((B, S, F), dtype=np.float32)
    for c in range(8):
        b, g = c // 2, c % 2
        out[b, :, g * G:(g + 1) * G] = results[c]["out"]
    return out


def run(X, residual_score, wq, wk, wv, trace=False):
    from concourse.bass_utils import run_bass_kernel_spmd

    nc = _get_nc()
    in_maps = _make_in_maps(X, residual_score, wq, wk, wv)
    res = run_bass_kernel_spmd(nc, in_maps, core_ids=list(range(8)), trace=trace)
    return _assemble(res.results), res


def kernel(X, residual_score, wq, wk, wv):
    out, _ = run(X, residual_score, wq, wk, wv)
    return (out, out)


# revision 6
# speedup vs baseline: 1.3467x; 1.3467x over previous
"""Multi-head QKV block attention for Trainium2, SPMD over 8 NeuronCores.

Problem: X[4,2048,1024], residual[4,2048,1024], wq/wk/wv[1024,1024],
H=16 heads, D=64, softmax scale sqrt(S/H)=sqrt(128).
out = softmax((X wq)(X wk)^T / sqrt(128)) (X wv) + residual, returned twice.

Sharding: core c handles batch b=c//2 and head group g=c%2 (8 heads = 512
feature columns). Fully data/tensor-parallel -- no collectives; host
assembles the output.

Per-core kernel (Tile framework):
  B) X s-tiles are PE-transposed to X^T (f32r); Q^T,K^T projected with
     weights as stationary operands (stored bf16), V projected in natural
     layout (f32r) with a ones-column appended per head.
  C) per head pair: transposed logits K_h^T.T @ Q_h^T as two K=64 matmuls
     in disjoint PE row groups; exp on ScalarE (scale folded, no max
     subtraction -- logits are O(10) so fp32 exp is safe); effect^T
     accumulated as [v|1].T @ expT (ones row gives the softmax denominator);
     PE-transpose back, normalize, add residual, DMA out.
"""

import math
import sys

for _p in ("/opt/trn_rl_repo", "/root/.axon_site/_ro/trn_rl_repo"):
    if _p not in sys.path:
        sys.path.append(_p)

import numpy as np

B, S, F = 4, 2048, 1024
H = 16
D = 64
G = 512            # feature columns per core (8 heads)
NH = 8             # heads per core
KC = 8             # contraction chunks of 128 over F
ST = 4             # s tiles of 512
TC = 16            # t chunks of 128
SCALE = 1.0 / math.sqrt(S / H)

_cached = None


def _build():
    import concourse.bacc as bacc
    import concourse.tile as tile
    from concourse import mybir
    from concourse.masks import make_identity

    dt = mybir.dt
    AF = mybir.ActivationFunctionType

    nc = bacc.Bacc("TRN2", target_bir_lowering=False, debug=False, num_devices=8)

    x_d = nc.dram_tensor("x", [S, F], dt.float32, kind="ExternalInput").ap()
    wq_d = nc.dram_tensor("wq", [F, G], dt.float32, kind="ExternalInput").ap()
    wk_d = nc.dram_tensor("wk", [F, G], dt.float32, kind="ExternalInput").ap()
    wv_d = nc.dram_tensor("wv", [F, G], dt.float32, kind="ExternalInput").ap()
    res_d = nc.dram_tensor("res", [S, G], dt.float32, kind="ExternalInput").ap()
    out_d = nc.dram_tensor("out", [S, G], dt.float32, kind="ExternalOutput").ap()

    with tile.TileContext(nc) as tc:
        with tc.tile_pool(name="persist", bufs=1) as persist:
            ident = persist.tile([128, 128], dt.float32)
            make_identity(nc, ident[:])
            ones = persist.tile([128, NH], dt.float32)
            nc.vector.memset(ones[:], 1.0)
            # Preload the exp table set on ScalarE while PE/DMA are busy with
            # phase B -- the ~2.7us ACT_TABLE_LOAD otherwise lands at the
            # phase boundary, idles PE past the HAM window, and the whole
            # attention phase runs at K=4/8 half clock.
            scr = persist.tile([128, NH], dt.float32)
            nc.scalar.activation(scr[:], ones[:], AF.Exp)
            qT = [persist.tile([128, S], dt.bfloat16, name=f"qT{m}") for m in range(4)]
            kT = [persist.tile([128, S], dt.bfloat16, name=f"kT{m}") for m in range(4)]
            vS = [persist.tile([128, NH, D + 1], dt.float32r, name=f"vS{t}")
                  for t in range(TC)]

            # ---- Phase B: transpose X, project Q^T, K^T, V ----
            with tc.tile_pool(name="wp", bufs=1) as wp, \
                 tc.tile_pool(name="xsp", bufs=6) as xsp, \
                 tc.tile_pool(name="xtp", bufs=9) as xtp, \
                 tc.tile_pool(name="psB", bufs=2, space="PSUM") as psB, \
                 tc.tile_pool(name="psP", bufs=2, space="PSUM") as psP:
                w_sb = {}
                for nm, wd in (("q", wq_d), ("k", wk_d), ("v", wv_d)):
                    for k in range(KC):
                        t = wp.tile([128, G], dt.float32r, name=f"w{nm}{k}")
                        nc.gpsimd.dma_start(t[:], wd[k * 128:(k + 1) * 128, :])
                        w_sb[nm, k] = t

                for st in range(ST):
                    s0 = st * 512
                    xs = []
                    for j in range(4):
                        t = xsp.tile([128, F], dt.float32, tag="xs", name="xs")
                        nc.sync.dma_start(t[:], x_d[s0 + j * 128:s0 + (j + 1) * 128, :])
                        xs.append(t)
                    xTs = []
                    for k in range(KC):
                        pt = psB.tile([128, 512], dt.float32, tag="xtp", name="pt")
                        for j in range(4):
                            nc.tensor.transpose(
                                pt[:, j * 128:(j + 1) * 128],
                                xs[j][:, k * 128:(k + 1) * 128], ident[:])
                        xt = xtp.tile([128, 512], dt.float32r, tag="xT", name="xt")
                        nc.vector.tensor_copy(xt[:], pt[:])
                        xTs.append(xt)
                    for m in range(4):
                        pk = psP.tile([128, 512], dt.float32, tag="proj", name="pk")
                        for k in range(KC):
                            nc.tensor.matmul(
                                pk[:], w_sb["k", k][:, m * 128:(m + 1) * 128],
                                xTs[k][:], start=(k == 0), stop=(k == KC - 1))
                        nc.vector.tensor_copy(kT[m][:, s0:s0 + 512], pk[:])
                    for m in range(4):
                        pq = psP.tile([128, 512], dt.float32, tag="proj", name="pq")
                        for k in range(KC):
                            nc.tensor.matmul(
                                pq[:], w_sb["q", k][:, m * 128:(m + 1) * 128],
                                xTs[k][:], start=(k == 0), stop=(k == KC - 1))
                        nc.vector.tensor_copy(qT[m][:, s0:s0 + 512], pq[:])
                    for j in range(4):
                        pv = psP.tile([128, 512], dt.float32, tag="proj", name="pv")
                        for k in range(KC):
                            nc.tensor.matmul(
                                pv[:], xTs[k][:, j * 128:(j + 1) * 128],
                                w_sb["v", k][:], start=(k == 0), stop=(k == KC - 1))
                        tci = st * 4 + j
                        nc.vector.tensor_copy(
                            vS[tci][:, :, D:D + 1],
                            ones[:].rearrange("p (h o) -> p h o", o=1))
                        nc.vector.tensor_copy(
                            vS[tci][:, :, 0:D],
                            pv[:].rearrange("p (h d) -> p h d", h=NH))

            # ---- Phase C: attention ----
            # lpp slots are [128,1024] fp32 = 2 PSUM banks; 3 bufs = 6 banks.
            # The epilogue transpose tiles share the lpp pool (disjoint in
            # time). epp = 2 banks. Total 8 banks.
            with tc.tile_pool(name="lpp", bufs=3, space="PSUM") as lpp, \
                 tc.tile_pool(name="epp", bufs=2, space="PSUM") as epp, \
                 tc.tile_pool(name="expp", bufs=3) as expp, \
                 tc.tile_pool(name="esp", bufs=2) as esp, \
                 tc.tile_pool(name="stp", bufs=8) as stp, \
                 tc.tile_pool(name="rsp", bufs=3) as rsp, \
                 tc.tile_pool(name="rcp", bufs=4) as rcp:
                for st in range(ST):
                    s0 = st * 512
                    stage = [stp.tile([128, G], dt.float32, tag="stage", name="stage")
                             for _ in range(4)]
                    for hp in range(4):
                        eps = [epp.tile([D + 1, 512], dt.float32, tag="ep", name="ep")
                               for _ in range(2)]
                        for tg in range(TC // 2):
                            t0 = tg * 2
                            # two K=64 logits matmuls on disjoint PE row
                            # groups run concurrently; batch 2 t-chunks per
                            # psum tile so exp runs as one [128,1024] op.
                            lps = [lpp.tile([128, 1024], dt.float32, tag="lp", name="lp")
                                   for _ in range(2)]
                            for i in range(2):
                                for half in range(2):
                                    r0 = half * 64
                                    t = t0 + i
                                    nc.tensor.matmul(
                                        lps[half][:, i * 512:(i + 1) * 512],
                                        kT[hp][r0:r0 + 64, t * 128:(t + 1) * 128],
                                        qT[hp][r0:r0 + 64, s0:s0 + 512],
                                        start=True, stop=True)
                            exs = []
                            for half in range(2):
                                ex = expp.tile([128, 1024], dt.float32r, tag="exp", name="ex")
                                nc.scalar.activation(ex[:], lps[half][:], AF.Exp, scale=SCALE)
                                exs.append(ex)
                            for i in range(2):
                                for half in range(2):
                                    nc.tensor.matmul(
                                        eps[half][:],
                                        vS[t0 + i][:, 2 * hp + half, :],
                                        exs[half][:, i * 512:(i + 1) * 512],
                                        start=(t0 + i == 0), stop=(t0 + i == TC - 1))
                        for half in range(2):
                            h = 2 * hp + half
                            es = esp.tile([D + 1, 512], dt.float32, tag="es", name="es")
                            nc.vector.tensor_copy(es[:], eps[half][:])
                            tp4 = lpp.tile([128, 4, D + 1], dt.float32, tag="lp", name="tp4")
                            for j in range(4):
                                nc.tensor.transpose(
                                    tp4[:, j, :], es[:, j * 128:(j + 1) * 128],
                                    ident[0:D + 1, 0:D + 1])
                            rec = rcp.tile([128, 4], dt.float32, tag="rec", name="rec")
                            nc.vector.reciprocal(rec[:], tp4[:, :, D])
                            for j in range(4):
                                nc.vector.tensor_scalar_mul(
                                    stage[j][:, h * 64:(h + 1) * 64],
                                    tp4[:, j, 0:D], rec[:, j:j + 1])
                    for j in range(4):
                        rt = rsp.tile([128, G], dt.float32, tag="res", name="rt")
                        nc.sync.dma_start(
                            rt[:], res_d[s0 + j * 128:s0 + (j + 1) * 128, :])
                        nc.vector.tensor_add(stage[j][:], stage[j][:], rt[:])
                        nc.sync.dma_start(
                            out_d[s0 + j * 128:s0 + (j + 1) * 128, :], stage[j][:])

    nc.compile()
    return nc


def _get_nc():
    global _cached
    if _cached is None:
        _cached = _build()
    return _cached


def _make_in_maps(X, residual_score, wq, wk, wv):
    X = np.ascontiguousarray(np.asarray(X, dtype=np.float32))
    residual_score = np.ascontiguousarray(np.asarray(residual_score, dtype=np.float32))
    wq = np.ascontiguousarray(np.asarray(wq, dtype=np.float32))
    wk = np.ascontiguousarray(np.asarray(wk, dtype=np.float32))
    wv = np.ascontiguousarray(np.asarray(wv, dtype=np.float32))
    in_maps = []
    for c in range(8):
        b, g = c // 2, c % 2
        cols = slice(g * G, (g + 1) * G)
        in_maps.append({
            "x": X[b],
            "wq": np.ascontiguousarray(wq[:, cols]),
            "wk": np.ascontiguousarray(wk[:, cols]),
            "wv": np.ascontiguousarray(wv[:, cols]),
            "res": np.ascontiguousarray(residual_score[b, :, cols]),
        })
    return in_maps


def _assemble(results):
    out = np.empty((B, S, F), dtype=np.float32)
    for c in range(8):
        b, g = c // 2, c % 2
        out[b, :, g * G:(g + 1) * G] = results[c]["out"]
    return out


def run(X, residual_score, wq, wk, wv, trace=False):
    from concourse.bass_utils import run_bass_kernel_spmd

    nc = _get_nc()
    in_maps = _make_in_maps(X, residual_score, wq, wk, wv)
    res = run_bass_kernel_spmd(nc, in_maps, core_ids=list(range(8)), trace=trace)
    return _assemble(res.results), res


def kernel(X, residual_score, wq, wk, wv):
    out, _ = run(X, residual_score, wq, wk, wv)
    return (out, out)
